# revision 16
# baseline (speedup 1.0000x reference)
"""Self-contained Trainium2 Bass kernel for causal multi-head attention.

Problem: y = Attention(x) with B=2, T=2048, C=1024, H=16 heads, HD=64,
causal softmax, fused qkv projection and output projection.

Sharding (8 NeuronCores): core c = (b, i) with b = c // 4 (data parallel on
batch), i = c % 4 (tensor parallel on heads: heads 4i..4i+3 and the matching
qkv columns / w_out rows+cols). Each core computes q/k transposed
(head-dim on partitions, tokens on free dim), v in natural layout, causal
softmax in the transposed domain, normalized y^T for its 4 heads (the
softmax normalizer Z rides along as a 65th ones-column on v), AllGathers
y^T across its 4-core group (one AllGather per head-pair, overlapped with
the second pair's attention), and computes a 256-column slice of the output
projection in two passes (one per gathered part). Host concatenates the
8 [2048, 256] slices.

All matmuls run as float32r (TF32-like) with fp32 PSUM accumulation.
The v-bias is folded into an effective output bias on the host
(softmax rows sum to 1, so y = attn@v + b_v exactly).
"""

import sys

sys.path.insert(0, "/opt/trn_rl_repo")

import numpy as np

B, T, C, H, HD = 2, 2048, 1024, 16, 64
P = 128
N_CORES = 8
GROUP = 4  # cores per batch == tensor-parallel group size
HPC = H // GROUP  # heads per core = 4
CPC = HPC * HD  # y/out columns per core = 256
QB = 512  # query block (free dim of attention matmuls)
NQB = T // QB  # 4
NCC = C // P  # 8 contraction chunks
NTT = T // P  # 16 token tiles
NPAIR = 2  # head pairs per core (2 heads each)

_cache: dict = {}


def _build_program(repeat: int = 1, single: bool = False):
    """Build + compile the per-core Bass program (same program on all cores)."""
    import concourse.bass as bass
    import concourse.mybir as mybir
    from concourse import bacc, tile

    f32 = mybir.dt.float32
    f32r = mybir.dt.float32r
    Exp = mybir.ActivationFunctionType.Exp
    Ident = mybir.ActivationFunctionType.Identity
    mult = mybir.AluOpType.mult
    add = mybir.AluOpType.add

    nc = bacc.Bacc("TRN2", target_bir_lowering=False, debug=False,
                   num_devices=N_CORES)

    xt = nc.dram_tensor("xt", [C, T], f32r, kind="ExternalInput").ap()
    wq0 = nc.dram_tensor("wq0", [C, P], f32r, kind="ExternalInput").ap()
    wq1 = nc.dram_tensor("wq1", [C, P], f32r, kind="ExternalInput").ap()
    wk0 = nc.dram_tensor("wk0", [C, P], f32r, kind="ExternalInput").ap()
    wk1 = nc.dram_tensor("wk1", [C, P], f32r, kind="ExternalInput").ap()
    wv = nc.dram_tensor("wv", [C, CPC], f32r, kind="ExternalInput").ap()
    wout = nc.dram_tensor("wout", [C, CPC], f32r, kind="ExternalInput").ap()
    bqk = nc.dram_tensor("bqk", [4, P], f32, kind="ExternalInput").ap()
    bout = nc.dram_tensor("bout", [CPC], f32, kind="ExternalInput").ap()
    out = nc.dram_tensor("out", [T, CPC], f32, kind="ExternalOutput").ap()

    xt_r = xt.rearrange("(o p) t -> p o t", p=P)  # [128, 8, 2048]

    with tile.TileContext(nc) as tc:
        import contextlib

        with contextlib.ExitStack() as ctx:
            const = ctx.enter_context(tc.tile_pool(name="const", bufs=1))
            wpool = ctx.enter_context(tc.tile_pool(name="wpool", bufs=1))
            xpool = ctx.enter_context(tc.tile_pool(name="xpool", bufs=2))
            slab = ctx.enter_context(tc.tile_pool(name="slab", bufs=1))
            work = ctx.enter_context(tc.tile_pool(name="work", bufs=3))
            psum = ctx.enter_context(tc.tile_pool(name="psum", bufs=2, space="PSUM"))
            dram = ctx.enter_context(tc.tile_pool(name="dram", bufs=1, space="DRAM"))

            # ---- constants ----
            ones1_64 = const.tile([1, 64], f32r)
            nc.vector.memset(ones1_64[:].bitcast(f32), 1.0)
            ones1 = const.tile([1, P], f32r)
            nc.vector.memset(ones1[:].bitcast(f32), 1.0)
            # causal mask strip: maskS[kp, u] = 1.0 iff u - kp - 384 >= 0.
            # chunk (qb, kc) with off = kc*128 - qb*512 in {0,128,256,384}
            # uses slice maskS[:, 384-off+c] for chunk column c.
            maskS = const.tile([P, 896], f32)
            nc.vector.memset(maskS[:], 1.0)
            nc.gpsimd.affine_select(
                out=maskS[:],
                in_=maskS[:],
                compare_op=mybir.AluOpType.is_ge,
                fill=0.0,
                base=-384,
                pattern=[[1, 896]],
                channel_multiplier=-1,
            )
            bout_sb = const.tile([1, CPC], f32r)
            bqk_sb = const.tile([P, 4], f32)

            # ---- q/k/v weights (w_out DMA is issued later, mid-attention) ----
            wq_sb = []
            for nm in ("wq0", "wq1", "wk0", "wk1"):
                wq_sb.append(wpool.tile([P, NCC, P], f32r, name=f"w_{nm}"))
            nc.sync.dma_start(wq_sb[0][:], wq0.rearrange("(o p) m -> p o m", p=P))
            nc.sync.dma_start(bqk_sb[:], bqk.rearrange("g p -> p g"))
            wv_sb = wpool.tile([P, NCC, CPC], f32r)
            wout_sb = wpool.tile([P, NCC, CPC], f32r)

            # ---- persistent slabs ----
            qt = [slab.tile([P, T], f32r, name=f"qt{i}") for i in range(NPAIR)]
            kt = [slab.tile([P, T], f32r, name=f"kt{i}") for i in range(NPAIR)]
            # v slab: per token-tile, HPC head slots of [64 v-dims | 1.0]
            # (the ones column folds the softmax normalizer Z into attn @ v)
            vsl = slab.tile([P, NTT, HPC, HD + 1], f32r)
            nc.vector.memset(
                vsl.rearrange("p t h x -> p (t h) x")[:, :, HD:HD + 1]
                .bitcast(f32), 1.0)
            ytsb = [slab.tile([P, T], f32, name=f"ytsb{i}") for i in range(NPAIR)]
            # partial out-proj results (pass A) awaiting the second gather
            opart = slab.tile([P, NTT, CPC], f32)

            for rep in range(repeat):
                # ---- projection interleaved with pair-0 attention ----
                # q/k transposed: out[hd-part, tok] = w_slice^T @ x^T
                # v natural:      out[tok-part, hd] = x @ w_v
                proj_groups = [
                    (wq_sb[0], bqk_sb[:, 0:1], qt[0]),
                    (wq_sb[2], bqk_sb[:, 2:3], kt[0]),
                    (wq_sb[1], bqk_sb[:, 1:2], qt[1]),
                    (wq_sb[3], bqk_sb[:, 3:4], kt[1]),
                ]

                def proj(qb, rep=rep):
                    qsl = slice(qb * QB, (qb + 1) * QB)
                    xc = xpool.tile([P, NCC, QB], f32r, tag="xc",
                                    name=f"xc_{rep}_{qb}")
                    for o in range(NCC):  # split so compute starts early
                        nc.sync.dma_start(xc[:, o, :], xt_r[:, o, qsl])
                    if rep == 0 and qb == 0:
                        # remaining weights load behind the first x chunk
                        nc.sync.dma_start(
                            wq_sb[2][:], wk0.rearrange("(o p) m -> p o m", p=P))
                        nc.sync.dma_start(
                            wv_sb[:], wv.rearrange("(o p) m -> p o m", p=P))
                        nc.sync.dma_start(
                            wq_sb[1][:], wq1.rearrange("(o p) m -> p o m", p=P))
                        nc.sync.dma_start(
                            wq_sb[3][:], wk1.rearrange("(o p) m -> p o m", p=P))
                    # pair-0 groups first so its attention unblocks early
                    for gi, (wsb, bcol, dest) in enumerate(proj_groups[:2]):
                        ps = psum.tile([P, QB], f32, tag="a",
                                       name=f"proj_{rep}_{qb}_{gi}")
                        for o in range(NCC):
                            nc.tensor.matmul(
                                ps[:], wsb[:, o, :], xc[:, o, :],
                                start=(o == 0), stop=(o == NCC - 1),
                            )
                        nc.scalar.activation(dest[:, qsl], ps[:], Ident,
                                             bias=bcol)
                    for tt in range(QB // P):
                        t0 = qb * (QB // P) + tt
                        pv = psum.tile([P, CPC], f32, tag="a",
                                       name=f"pv_{rep}_{t0}")
                        for o in range(NCC):
                            nc.tensor.matmul(
                                pv[:], xc[:, o, tt * P:(tt + 1) * P],
                                wv_sb[:, o, :],
                                start=(o == 0), stop=(o == NCC - 1),
                            )
                        nc.scalar.copy(
                            out=vsl[:, t0, :, 0:HD],
                            in_=pv.rearrange("p (h x) -> p h x", h=HPC))
                    for gi, (wsb, bcol, dest) in enumerate(proj_groups[2:]):
                        ps = psum.tile([P, QB], f32, tag="a",
                                       name=f"proj_{rep}_{qb}_{2 + gi}")
                        for o in range(NCC):
                            nc.tensor.matmul(
                                ps[:], wsb[:, o, :], xc[:, o, :],
                                start=(o == 0), stop=(o == NCC - 1),
                            )
                        nc.scalar.activation(dest[:, qsl], ps[:], Ident,
                                             bias=bcol)

                def attn(pair, qb, rep=rep):
                    qsl0 = qb * QB
                    ytp = [psum.tile([P, QB], f32, tag="yt", bufs=3,
                                     name=f"yt_{rep}_{pair}_{qb}_{h}")
                           for h in range(2)]
                    nkc = (qb + 1) * (QB // P)
                    for kc in range(nkc):
                        off = kc * P - qb * QB
                        diag = off >= 0
                        # narrow diagonal chunks: columns < estart are fully
                        # masked; [estart, off+128) is the partial band
                        # (min width 256 keeps f32r at full rate)
                        es = min(off, QB - 256) if diag else 0
                        sc = psum.tile([P, 2, QB], f32, tag="a",
                                       name=f"sc_{rep}_{pair}_{qb}_{kc}")
                        for h in range(2):
                            hp = slice(h * 64, (h + 1) * 64)
                            nc.tensor.matmul(
                                sc[:, h, es:],
                                kt[pair][hp, kc * P:(kc + 1) * P],
                                qt[pair][hp, qsl0 + es:qsl0 + QB],
                                start=True, stop=True,
                                tile_position=(h * 64, 0),
                                skip_group_check=True,
                            )
                        et = work.tile([P, 2, QB], f32r, tag="et", bufs=4,
                                       name=f"et_{rep}_{pair}_{qb}_{kc}")
                        nc.scalar.activation(et[:, :, es:], sc[:, :, es:],
                                             Exp, scale=0.125)
                        if diag:  # zero the non-causal band
                            be = min(off + P, QB)
                            for h in range(2):
                                nc.vector.tensor_tensor(
                                    et[:, h, es:be], et[:, h, es:be],
                                    maskS[:, 384 - off + es:384 - off + be],
                                    mult)
                        first, last = kc == 0, kc == nkc - 1
                        for h in range(2):
                            # [v | 1] lhsT: row 64 of the output is Z
                            nc.tensor.matmul(
                                ytp[h][0:HD + 1, es:],
                                vsl[:, kc, pair * 2 + h, :],
                                et[:, h, es:],
                                start=first, stop=last,
                                skip_group_check=True,
                            )
                    for h in range(2):
                        zi = work.tile([1, QB], f32r, tag="zi",
                                       name=f"zi_{rep}_{pair}_{qb}_{h}")
                        with nc.allow_low_precision(
                                reason="f32r zinv feeds replicate matmul"):
                            nc.vector.reciprocal(zi[:], ytp[h][HD:HD + 1, :])
                        zr = psum.tile([HD, QB], f32, tag="zr", bufs=1,
                                       name=f"zr_{rep}_{pair}_{qb}_{h}")
                        nc.tensor.matmul(zr[:], ones1_64[:], zi[:],
                                         start=True, stop=True,
                                         skip_group_check=True)
                        zrs = work.tile([HD, QB], f32, tag="zrs",
                                        name=f"zrs_{rep}_{pair}_{qb}_{h}")
                        nc.vector.tensor_copy(out=zrs[:], in_=zr[:])
                        nc.vector.tensor_tensor(
                            ytsb[pair][h * HD:(h + 1) * HD, qsl0:qsl0 + QB],
                            ytp[h][0:HD, :], zrs[:], mult)

                def gather(pair, rep=rep):
                    # pair 0's gather overlaps pair 1's attention; rank r
                    # contributes heads (4r + 2*pair, +1)
                    ytl = dram.tile([P, T], f32, name=f"ytl_{rep}_{pair}")
                    nc.sync.dma_start(ytl[:], ytsb[pair][:])
                    ytfp = dram.tile([GROUP * P, T], f32,
                                     name=f"ytf_{rep}_{pair}")
                    if single:
                        for g in range(GROUP):  # timing stand-in for the AG
                            nc.sync.dma_start(
                                ytfp[g * P:(g + 1) * P, :], ytl[:])
                    else:
                        nc.gpsimd.collective_compute(
                            "AllGather",
                            mybir.AluOpType.bypass,
                            replica_groups=[[0, 1, 2, 3], [4, 5, 6, 7]],
                            ins=[ytl.opt()],
                            outs=[ytfp.opt()],
                        )
                    return ytfp.rearrange("(o p) t -> p o t", p=P)

                # projection interleaved with pair-0 attention per q-block
                for qb in range(NQB):
                    proj(qb)
                    attn(0, qb)
                ytf = [gather(0)]
                # w_out (rows permuted on host to gather order) is first
                # needed by out-proj pass A, mid pair-1 attention
                nc.sync.dma_start(
                    wout_sb[:], wout.rearrange("(o p) m -> p o m", p=P))
                nc.sync.dma_start(
                    bout_sb[:],
                    bout.rearrange("(o m) -> o m", o=1).bitcast(f32r))
                for qb in range(NQB):
                    attn(1, qb)
                ytf.append(gather(1))

                # ---- output projection (column slice), two passes:
                # pass A consumes gather 0 while gather 1 is still in flight
                ytt0s = []
                for t4 in range(NTT // 4):
                    ytt0 = work.tile([P, GROUP, 4 * P], f32r, tag="ytt", bufs=2,
                                     name=f"ytt0_{rep}_{t4}")
                    nc.sync.dma_start(
                        ytt0[:],
                        ytf[0][:, :, t4 * 4 * P:(t4 + 1) * 4 * P].bitcast(f32r))
                    ytt0s.append(ytt0)
                for tt in range(NTT):
                    ytt0 = ytt0s[tt // 4][:, :, (tt % 4) * P:(tt % 4 + 1) * P]
                    po = psum.tile([P, CPC], f32, tag="a",
                                   name=f"poA_{rep}_{tt}")
                    # bias init via rank-1 ones matmul, then accumulate
                    nc.tensor.matmul(po[:], ones1[:], bout_sb[:],
                                     start=True, stop=False,
                                     skip_group_check=True)
                    for o in range(GROUP):
                        nc.tensor.matmul(
                            po[:], ytt0[:, o, :], wout_sb[:, o, :],
                            start=False, stop=(o == GROUP - 1),
                            skip_group_check=True,
                        )
                    nc.vector.tensor_copy(out=opart[:, tt, :], in_=po[:])
                ytt1g = None
                osb4 = None
                for tt in range(NTT):
                    if tt % 4 == 0:
                        t4 = tt // 4
                        ytt1g = work.tile([P, GROUP, 4 * P], f32r, tag="ytt",
                                          bufs=2, name=f"ytt1_{rep}_{t4}")
                        nc.sync.dma_start(
                            ytt1g[:],
                            ytf[1][:, :, t4 * 4 * P:(t4 + 1) * 4 * P]
                            .bitcast(f32r))
                    ytt1 = ytt1g[:, :, (tt % 4) * P:(tt % 4 + 1) * P]
                    po = psum.tile([P, CPC], f32, tag="a",
                                   name=f"poB_{rep}_{tt}")
                    for o in range(GROUP):
                        nc.tensor.matmul(
                            po[:], ytt1[:, o, :], wout_sb[:, GROUP + o, :],
                            start=(o == 0), stop=(o == GROUP - 1),
                            skip_group_check=True,
                        )
                    if tt % 4 == 0:
                        osb4 = work.tile([P, 4, CPC], f32, tag="osb", bufs=2,
                                         name=f"osb_{rep}_{tt // 4}")
                    nc.vector.tensor_tensor(osb4[:, tt % 4, :], po[:],
                                            opart[:, tt, :], add)
                    if tt % 4 == 3:
                        t4 = tt // 4
                        nc.sync.dma_start(
                            out[t4 * 4 * P:(t4 + 1) * 4 * P, :]
                            .rearrange("(tb p) m -> p tb m", p=P), osb4[:])

    nc.compile()
    return nc


def _get_program(repeat: int = 1, single: bool = False):
    key = ("nc", repeat, single)
    if key not in _cache:
        _cache[key] = _build_program(repeat, single)
    return _cache[key]


def prepare_in_maps(x, w_qkv, b_qkv, w_out, b_out):
    """Shard full inputs into the 8 per-core input maps."""
    x = np.asarray(x, dtype=np.float32)
    w_qkv = np.asarray(w_qkv, dtype=np.float32)
    b_qkv = np.asarray(b_qkv, dtype=np.float32)
    w_out = np.asarray(w_out, dtype=np.float32)
    b_out = np.asarray(b_out, dtype=np.float32)

    xts = [np.ascontiguousarray(x[b].T) for b in range(B)]
    # softmax rows sum to 1 => y = attn @ v + b_v exactly, so the v-bias
    # folds into an effective output bias on the host
    b_out_eff = (b_out.astype(np.float64)
                 + b_qkv[2 * C:].astype(np.float64) @ w_out.astype(np.float64)
                 ).astype(np.float32)

    in_maps = []
    for c in range(N_CORES):
        b, i = divmod(c, GROUP)
        h0 = i * HPC  # first head of this core
        qc = slice(h0 * HD, (h0 + HPC) * HD)  # 256 q columns
        q0 = slice(h0 * HD, h0 * HD + 2 * HD)  # first head pair (128 cols)
        q1 = slice(h0 * HD + 2 * HD, (h0 + HPC) * HD)
        wout_cols = w_out[:, i * CPC:(i + 1) * CPC]
        # row order must match the per-pair AllGather layout:
        # part p rows = [rank r, pair p (128 rows) for r in 0..3]
        wout_perm = np.concatenate(
            [wout_cols[r * CPC:r * CPC + P] for r in range(GROUP)]
            + [wout_cols[r * CPC + P:(r + 1) * CPC] for r in range(GROUP)])
        in_maps.append({
            "xt": xts[b],
            "wq0": np.ascontiguousarray(w_qkv[:, q0]),
            "wq1": np.ascontiguousarray(w_qkv[:, q1]),
            "wk0": np.ascontiguousarray(w_qkv[:, C + q0.start: C + q0.stop]),
            "wk1": np.ascontiguousarray(w_qkv[:, C + q1.start: C + q1.stop]),
            "wv": np.ascontiguousarray(w_qkv[:, 2 * C + qc.start: 2 * C + qc.stop]),
            "wout": np.ascontiguousarray(wout_perm),
            "bqk": np.ascontiguousarray(np.stack([
                b_qkv[q0], b_qkv[q1],
                b_qkv[C + q0.start: C + q0.stop],
                b_qkv[C + q1.start: C + q1.stop]])),
            "bout": np.ascontiguousarray(b_out_eff[i * CPC:(i + 1) * CPC]),
        })
    return in_maps


def run_device(in_maps, repeat: int = 1):
    """Execute the compiled SPMD program; returns per-core result dicts.

    The NeuronCores occasionally come up wedged (NRT_EXEC_UNIT_UNRECOVERABLE
    / LoadExecutable failures) if a previous process died mid-execution;
    they recover after a short wait, so retry with backoff.
    """
    import time as _time
    from concourse import bass_utils

    nc = _get_program(repeat)
    last_err = None
    for attempt in range(3):
        try:
            res = bass_utils.run_bass_kernel_spmd(
                nc, in_maps, core_ids=list(range(N_CORES)))
            return res.results
        except Exception as e:  # device wedge: wait for recovery and retry
            last_err = e
            if attempt < 2:
                _time.sleep(75)
    raise last_err


def assemble_output(results):
    out = np.empty((B, T, C), dtype=np.float32)
    for c in range(N_CORES):
        b, i = divmod(c, GROUP)
        out[b, :, i * CPC:(i + 1) * CPC] = results[c]["out"]
    return out


def kernel(x, w_qkv, b_qkv, w_out, b_out):
    in_maps = prepare_in_maps(x, w_qkv, b_qkv, w_out, b_out)
    results = run_device(in_maps)
    return assemble_output(results)


if __name__ == "__main__":
    rng = np.random.default_rng(0)
    inputs = {
        "x": rng.standard_normal((B, T, C), dtype=np.float32),
        "w_qkv": rng.standard_normal((C, 3 * C), dtype=np.float32) / np.sqrt(C),
        "b_qkv": rng.standard_normal(3 * C, dtype=np.float32) * 0.1,
        "w_out": rng.standard_normal((C, C), dtype=np.float32) / np.sqrt(C),
        "b_out": rng.standard_normal(C, dtype=np.float32) * 0.1,
    }
    y = kernel(**inputs)
    print("kernel output:", y.shape, y.dtype, float(np.abs(y).max()))
